# revision 8
# baseline (speedup 1.0000x reference)
"""MoE-routed dynamics MLP on 8 NeuronCores.

Expert-parallel: core p holds expert p's weights. Samples are dispatched
host-side (sort by policy index), each core runs its ~B/P samples through
  concat(latent, action) [C,528] -> H=1024 (relu) -> H=1024 (relu) -> 512
with activations kept transposed ([feature, sample]) so the three GEMMs
chain on the PE without any on-chip transposes:
  h1T = relu(W1.T @ xT + b1),  h2T = relu(W2.T @ h1T + b2),
  outT = W3.T @ h2T + b3.

Everything streams in bf16 (weights, x, inter-layer activations, output;
fp32 PSUM accumulate + fp32 bias): halves HBM traffic vs f32/f32r and
enables Fast Weight Load (LDWEIGHTS ~53ns, hidden behind the ~115ns
matmuls; f32r blocks FWL and exposes ~25ns/matmul). Accuracy budget is
~3e-3 vs the 2e-2 gate.

The sample dim C (max per-expert count, padded) is split into n-chunks
of <=512 (PSUM bank limit). Pass structure keeps the PE dense:
  - L1 n-chunk 0: K-chunk-outer / M-tile-inner, consuming each w1 K-chunk
    the moment its DMA lands (8 PSUM banks live).
  - every other pass (L1 chunk 1+, L2, L3): M-tile-outer / K-inner, so
    each M-tile's PSUM->SBUF eviction (bias+relu, alternating ScalarE/
    VectorE) hides behind the next M-tile's K-run, and each pass's rhs
    was already evicted during the previous pass -- no inter-layer stalls.
w2/w3 are pre-tiled M-chunk-major so an M-outer pass only needs its own
chunk's DMA (JIT for L2 as well). Input DMAs are issued on BOTH HWDGE
queues (sync: x/w1/bias, scalar: w2/w3) to halve serial issue cost
(~645ns per dma_start). A short bf16 warmup block heats the PE clock
gate (HAM, 1.2->2.4GHz after ~3.4us of sustained activity) while the
first chunks stream in. Output is evicted per (n-chunk, M-tile) to bf16
and DMA'd out immediately; the host upcasts to f32.
"""

import numpy as np
import ml_dtypes

BF16 = ml_dtypes.bfloat16

P = 8
D_LAT = 512
D_ACT = 16
D_IN = D_LAT + D_ACT  # 528
D_IN_PAD = 640        # 5 x 128
H = 1024
B = 4096

_compiled = {}  # capacity -> nc

# Results of the last run_bass_kernel_spmd call (for external harnesses
# that want exec_time_ns when tracing is enabled via BASS_TRACE).
LAST_RESULT = None


def _pretile(a):
    """[(k*128), F] row-major -> [128, k*F] partition-major chunks."""
    k = a.shape[0] // 128
    f = a.shape[1]
    return np.ascontiguousarray(
        a[: k * 128].reshape(k, 128, f).transpose(1, 0, 2).reshape(128, k * f)
    )


def _n_slices(C):
    """Split the sample dim into <=512 equal chunks (PSUM bank limit)."""
    k = -(-C // 512)
    base, rem = divmod(C, k)
    sizes = [base + (1 if i < rem else 0) for i in range(k)]
    out = []
    off = 0
    for s in sizes:
        out.append((off, s))
        off += s
    return out


def _build(C):
    import concourse.bacc as bacc
    import concourse.mybir as mybir
    import concourse.tile as tile

    f32 = mybir.dt.float32
    bf16 = mybir.dt.bfloat16
    AF = mybir.ActivationFunctionType
    ALU = mybir.AluOpType

    nc = bacc.Bacc(None, target_bir_lowering=False)

    xn = nc.declare_dram_parameter("xn", [128, 5 * C], bf16, isOutput=False)
    w1 = nc.declare_dram_parameter("w1", [128, 5 * H], bf16, isOutput=False)
    bias = nc.declare_dram_parameter("bias", [128, 20], f32, isOutput=False)
    w2 = nc.declare_dram_parameter("w2", [128, 8 * H], bf16, isOutput=False)
    w3 = nc.declare_dram_parameter("w3", [128, 4 * H], bf16, isOutput=False)
    ot = nc.declare_dram_parameter("ot", [128, 4 * C], bf16, isOutput=True)

    m1 = H // 128      # 8 M-tiles for layers 1/2
    m3 = D_LAT // 128  # 4 M-tiles for layer 3
    ns = _n_slices(C)
    nj = len(ns)
    nsz = ns[0][1]

    with tile.TileContext(nc) as tc:
        with (
            tc.tile_pool(name="xw", bufs=1) as xw,
            tc.tile_pool(name="acts", bufs=1) as acts,
            tc.tile_pool(name="ps0", bufs=1, space="PSUM") as ps_0,
            tc.tile_pool(name="ps1", bufs=1, space="PSUM") as ps_1,
            tc.tile_pool(name="ps2", bufs=1, space="PSUM") as ps_2,
            tc.tile_pool(name="ps3", bufs=1, space="PSUM") as ps_3,
            tc.tile_pool(name="ps4", bufs=1, space="PSUM") as ps_4,
            tc.tile_pool(name="ps5", bufs=1, space="PSUM") as ps_5,
            tc.tile_pool(name="ps6", bufs=1, space="PSUM") as ps_6,
            tc.tile_pool(name="ps7", bufs=1, space="PSUM") as ps_7,
        ):
            # One PSUM pool (= one 2KB bank) per M-tile index: bank m is
            # cycled L1(j0,m) -> L1(j1,m) -> L2(j0,m) -> L2(j1,m) -> L3(m),
            # so each reuse waits on exactly the eviction of the SAME m one
            # full pass earlier (always long done) instead of whichever slot
            # a shared pool happens to hand out.
            psp = [ps_0, ps_1, ps_2, ps_3, ps_4, ps_5, ps_6, ps_7]
            # Warmup operands (Tile requires every read tile to be written).
            wu_s = xw.tile([128, 128], bf16, name="wu_s")
            nc.vector.memset(wu_s[:], 0.0)
            wu_m = xw.tile([128, 512], bf16, name="wu_m")
            nc.vector.memset(wu_m[:], 0.0)

            # --- input DMAs -------------------------------------------------
            # ALL bandwidth-heavy inputs go on ONE queue (sync), strictly in
            # consumption order: the 16 SDMA engines round-robin between
            # queues at packet granularity, so a second busy queue steals
            # bandwidth from the JIT-critical stream. Each dma_start costs
            # ~0.6us issue + ~1.3us completion-receipt before its semaphore
            # fires, so x(j=0) is split per K-chunk for an early L1 start.
            # bias rides the scalar queue (one issue; queue then stays free
            # for evictions).
            bias_t = xw.tile([128, 20], f32, name="bias_t")
            nc.scalar.dma_start(out=bias_t[:], in_=bias[:])
            xn_t = [None] * nj
            xn_t[0] = xw.tile([128, 5 * nsz], bf16, name="xn_0")
            nc.sync.dma_start(out=xn_t[0][:, :nsz], in_=xn[:, :nsz])
            w1_t = []
            t = xw.tile([128, H], bf16, name="w1_0")
            nc.sync.dma_start(out=t[:], in_=w1[:, :H])
            w1_t.append(t)
            nc.sync.dma_start(
                out=xn_t[0][:, nsz : 2 * nsz], in_=xn[:, nsz : 2 * nsz]
            )
            t = xw.tile([128, H], bf16, name="w1_1")
            nc.sync.dma_start(out=t[:], in_=w1[:, H : 2 * H])
            w1_t.append(t)
            nc.sync.dma_start(
                out=xn_t[0][:, 2 * nsz : 5 * nsz], in_=xn[:, 2 * nsz : 5 * nsz]
            )
            for k in range(2, 5):
                t = xw.tile([128, H], bf16, name=f"w1_{k}")
                nc.sync.dma_start(out=t[:], in_=w1[:, k * H : (k + 1) * H])
                w1_t.append(t)
            for j in range(1, nj):
                xn_t[j] = xw.tile([128, 5 * nsz], bf16, name=f"xn_{j}")
                nc.sync.dma_start(
                    out=xn_t[j][:],
                    in_=xn[:, j * 5 * nsz : (j + 1) * 5 * nsz],
                )
            w2_t = []
            for i in range(4):
                t = xw.tile([128, 2 * H], bf16, name=f"w2_{i}")
                nc.sync.dma_start(out=t[:], in_=w2[:, i * 2 * H : (i + 1) * 2 * H])
                w2_t.append(t)
            w3_t = []
            for i in range(2):
                t = xw.tile([128, 2 * H], bf16, name=f"w3_{i}")
                nc.sync.dma_start(out=t[:], in_=w3[:, i * 2 * H : (i + 1) * 2 * H])
                w3_t.append(t)

            def w2_at(m, k):
                return w2_t[m // 2][
                    :, (m % 2) * H + k * 128 : (m % 2) * H + (k + 1) * 128
                ]

            def w3_at(m, k):
                return w3_t[m // 2][
                    :, (m % 2) * H + k * 128 : (m % 2) * H + (k + 1) * 128
                ]

            # Warmup: bf16 matmuls with no data dependencies heat the PE
            # clock gate (HAM) while the first chunks stream in.
            wu_p = psp[7].tile([128, 512], f32, tag="b7", name="wu_p")
            for _ in range(8):
                nc.tensor.matmul(
                    wu_p[:], lhsT=wu_s[:], rhs=wu_m[:], start=True, stop=True
                )

            # Inter-layer tiles are split per (n-chunk, M-tile) so consumers
            # depend only on the slice actually written (Tile tracks deps at
            # tile granularity).
            h1_t = [
                [acts.tile([128, nsz], bf16, name=f"h1_{j}_{m}") for m in range(m1)]
                for j in range(nj)
            ]
            h2_t = [
                [acts.tile([128, nsz], bf16, name=f"h2_{j}_{m}") for m in range(m1)]
                for j in range(nj)
            ]
            o_t = [
                [acts.tile([128, nsz], bf16, name=f"o_{j}_{m}") for m in range(m3)]
                for j in range(nj)
            ]

            ev_n = [0]

            def evict(out_ap, ps, bias_col, relu):
                """PSUM->SBUF eviction with bias (+relu), alternating
                ScalarE / VectorE so evictions never pace the PE."""
                b = bias_t[:, bias_col : bias_col + 1]
                if ev_n[0] % 2 == 0:
                    nc.scalar.activation(
                        out_ap, ps, AF.Relu if relu else AF.Identity, bias=b
                    )
                else:
                    if relu:
                        nc.vector.tensor_scalar(
                            out_ap, ps, b, 0.0, ALU.add, ALU.max
                        )
                    else:
                        nc.vector.tensor_scalar_add(out_ap, ps, b)
                ev_n[0] += 1

            # --- L1, n-chunk 0: K-outer / M-inner (JIT on w1 chunks) -------
            ps1 = [
                psp[m].tile([128, nsz], f32, tag=f"b{m}", name=f"ps1_{m}")
                for m in range(m1)
            ]
            for k in range(5):
                for m in range(m1):
                    nc.tensor.matmul(
                        ps1[m][:],
                        lhsT=w1_t[k][:, m * 128 : (m + 1) * 128],
                        rhs=xn_t[0][:, k * nsz : (k + 1) * nsz],
                        start=(k == 0),
                        stop=(k == 4),
                    )
            for m in range(m1):
                evict(h1_t[0][m][:], ps1[m][:], m, True)

            # --- L1, n-chunks 1+: M-outer / K-inner ------------------------
            for j in range(1, nj):
                for m in range(m1):
                    ps = psp[m].tile([128, nsz], f32, tag=f"b{m}", name=f"psa_{j}_{m}")
                    for k in range(5):
                        nc.tensor.matmul(
                            ps[:],
                            lhsT=w1_t[k][:, m * 128 : (m + 1) * 128],
                            rhs=xn_t[j][:, k * nsz : (k + 1) * nsz],
                            start=(k == 0),
                            stop=(k == 4),
                        )
                    evict(h1_t[j][m][:], ps[:], m, True)

            # --- L2: M-outer / K-inner per n-chunk -------------------------
            for j in range(nj):
                for m in range(m1):
                    ps = psp[m].tile([128, nsz], f32, tag=f"b{m}", name=f"psb_{j}_{m}")
                    for k in range(m1):
                        nc.tensor.matmul(
                            ps[:],
                            lhsT=w2_at(m, k),
                            rhs=h1_t[j][k][:],
                            start=(k == 0),
                            stop=(k == m1 - 1),
                        )
                    evict(h2_t[j][m][:], ps[:], 8 + m, True)

            # --- L3: M-outer / K-inner, flush each output immediately ------
            for j in range(nj):
                n0 = ns[j][0]
                for m in range(m3):
                    ps = psp[m].tile([128, nsz], f32, tag=f"b{m}", name=f"psc_{j}_{m}")
                    for k in range(m1):
                        nc.tensor.matmul(
                            ps[:],
                            lhsT=w3_at(m, k),
                            rhs=h2_t[j][k][:],
                            start=(k == 0),
                            stop=(k == m1 - 1),
                        )
                    evict(o_t[j][m][:], ps[:], 16 + m, False)
                    nc.sync.dma_start(
                        out=ot[:, m * C + n0 : m * C + n0 + nsz],
                        in_=o_t[j][m][:],
                    )

    nc.compile()
    return nc


def _ensure_axon_hooks():
    """run_bass_kernel_spmd(trace=True) imports antenv.axon_hooks, which the
    slim container lacks; provide it so tracing (e.g. BASS_TRACE=1) degrades
    gracefully or, if the ctypes hook is available, works."""
    import sys
    import types

    try:
        import antenv.axon_hooks  # noqa: F401
        return
    except ImportError:
        pass
    m = types.ModuleType("antenv.axon_hooks")
    m._hook = None
    m.set_axon_ntff_profile_hook = lambda h: setattr(m, "_hook", h)
    m.get_axon_ntff_profile_hook = lambda: m._hook
    sys.modules["antenv.axon_hooks"] = m
    try:
        from trn_agent_boot.trn_boot import _ntff_profile_via_ctypes

        m.set_axon_ntff_profile_hook(
            _ntff_profile_via_ctypes("/opt/axon/libaxon_pjrt.so")
        )
    except Exception:
        pass


def kernel(latents, actions, policy_indices, W1, b1, W2, b2, W3, b3):
    global LAST_RESULT
    _ensure_axon_hooks()
    from concourse.bass_utils import run_bass_kernel_spmd

    latents = np.ascontiguousarray(np.asarray(latents, dtype=np.float32))
    actions = np.ascontiguousarray(np.asarray(actions, dtype=np.float32))
    idx = np.asarray(policy_indices).astype(np.int64)
    W1 = np.asarray(W1, dtype=np.float32)
    b1 = np.asarray(b1, dtype=np.float32)
    W2 = np.asarray(W2, dtype=np.float32)
    b2 = np.asarray(b2, dtype=np.float32)
    W3 = np.asarray(W3, dtype=np.float32)
    b3 = np.asarray(b3, dtype=np.float32)

    n = latents.shape[0]
    order = np.argsort(idx, kind="stable")
    counts = np.bincount(idx, minlength=P)

    C = max(512, int(-(-counts.max() // 32)) * 32)
    k = -(-C // 512)
    C = -(-C // (16 * k)) * (16 * k)  # equal n-slices, width multiple of 16
    if C not in _compiled:
        _compiled[C] = _build(C)
    nc = _compiled[C]

    x = np.concatenate([latents, actions], axis=1)  # [B, 528]

    in_maps = []
    starts = np.concatenate([[0], np.cumsum(counts)])
    nsl = _n_slices(C)
    for p in range(P):
        sel = order[starts[p] : starts[p + 1]]
        xp = np.zeros((D_IN_PAD, C), dtype=BF16)
        xp[:D_IN, : counts[p]] = np.ascontiguousarray(x[sel].T).astype(BF16)
        xnp = np.concatenate(
            [_pretile(xp[:, n0 : n0 + nsz]) for n0, nsz in nsl], axis=1
        )
        w1r = np.zeros((D_IN_PAD, H), dtype=BF16)
        w1r[:D_IN] = W1[p].astype(BF16)
        w2p = np.concatenate(
            [_pretile(W2[p][:, m * 128 : (m + 1) * 128].astype(BF16))
             for m in range(8)],
            axis=1,
        )
        w3p = np.concatenate(
            [_pretile(W3[p][:, m * 128 : (m + 1) * 128].astype(BF16))
             for m in range(4)],
            axis=1,
        )
        bp = np.concatenate(
            [
                b1[p].reshape(H // 128, 128).T,
                b2[p].reshape(H // 128, 128).T,
                b3[p].reshape(D_LAT // 128, 128).T,
            ],
            axis=1,
        )
        in_maps.append(
            {
                "xn": xnp,
                "w1": _pretile(w1r),
                "bias": np.ascontiguousarray(bp),
                "w2": w2p,
                "w3": w3p,
            }
        )

    res = run_bass_kernel_spmd(nc, in_maps, core_ids=list(range(P)))
    LAST_RESULT = res

    out = np.empty((n, D_LAT), dtype=np.float32)
    for p in range(P):
        sel = order[starts[p] : starts[p + 1]]
        # [128, 4, C] -> [D_LAT, C]
        op = (
            res.results[p]["ot"]
            .reshape(128, 4, C)
            .transpose(1, 0, 2)
            .reshape(D_LAT, C)
            .astype(np.float32)
        )
        out[sel] = op[:, : counts[p]].T
    return out


# revision 10
# speedup vs baseline: 1.1402x; 1.1402x over previous
"""MoE-routed dynamics MLP on 8 NeuronCores.

Expert-parallel: core p holds expert p's weights. Samples are dispatched
host-side (sort by policy index), each core runs its ~B/P samples through
  concat(latent, action) [C,528] -> H=1024 (relu) -> H=1024 (relu) -> 512
with activations kept transposed ([feature, sample]) so the three GEMMs
chain on the PE without any on-chip transposes:
  h1T = relu(W1.T @ xT + b1),  h2T = relu(W2.T @ h1T + b2),
  outT = W3.T @ h2T + b3.

Everything streams in bf16 (weights, x, inter-layer activations, output;
fp32 PSUM accumulate + fp32 bias): halves HBM traffic vs f32/f32r and
enables Fast Weight Load (LDWEIGHTS ~53ns, hidden behind the ~115ns
matmuls; f32r blocks FWL and exposes ~25ns/matmul). Accuracy budget is
~3e-3 vs the 2e-2 gate.

The sample dim C (max per-expert count, padded) is split into n-chunks
of <=512 (PSUM bank limit). Pass structure keeps the PE dense:
  - L1 n-chunk 0: K-chunk-outer / M-tile-inner, consuming each w1 K-chunk
    the moment its DMA lands (8 PSUM banks live).
  - every other pass (L1 chunk 1+, L2, L3): M-tile-outer / K-inner, so
    each M-tile's PSUM->SBUF eviction (bias+relu, alternating ScalarE/
    VectorE) hides behind the next M-tile's K-run, and each pass's rhs
    was already evicted during the previous pass -- no inter-layer stalls.
w2/w3 are pre-tiled M-chunk-major so an M-outer pass only needs its own
chunk's DMA (JIT for L2 as well). Input DMAs are issued on BOTH HWDGE
queues (sync: x/w1/bias, scalar: w2/w3) to halve serial issue cost
(~645ns per dma_start). A short bf16 warmup block heats the PE clock
gate (HAM, 1.2->2.4GHz after ~3.4us of sustained activity) while the
first chunks stream in. Output is evicted per (n-chunk, M-tile) to bf16
and DMA'd out immediately; the host upcasts to f32.
"""

import numpy as np
import ml_dtypes

BF16 = ml_dtypes.bfloat16

P = 8
D_LAT = 512
D_ACT = 16
D_IN = D_LAT + D_ACT  # 528
D_IN_PAD = 640        # 5 x 128
H = 1024
B = 4096

_compiled = {}  # capacity -> nc

# Results of the last run_bass_kernel_spmd call (for external harnesses
# that want exec_time_ns when tracing is enabled via BASS_TRACE).
LAST_RESULT = None


def _pretile(a):
    """[(k*128), F] row-major -> [128, k*F] partition-major chunks."""
    k = a.shape[0] // 128
    f = a.shape[1]
    return np.ascontiguousarray(
        a[: k * 128].reshape(k, 128, f).transpose(1, 0, 2).reshape(128, k * f)
    )


def _n_slices(C):
    """Split the sample dim into <=512 equal chunks (PSUM bank limit)."""
    k = -(-C // 512)
    base, rem = divmod(C, k)
    sizes = [base + (1 if i < rem else 0) for i in range(k)]
    out = []
    off = 0
    for s in sizes:
        out.append((off, s))
        off += s
    return out


def _build(C):
    import concourse.bacc as bacc
    import concourse.mybir as mybir
    import concourse.tile as tile

    f32 = mybir.dt.float32
    bf16 = mybir.dt.bfloat16
    AF = mybir.ActivationFunctionType
    ALU = mybir.AluOpType

    nc = bacc.Bacc(None, target_bir_lowering=False)

    xn = nc.declare_dram_parameter("xn", [128, 5 * C], bf16, isOutput=False)
    w1 = nc.declare_dram_parameter("w1", [128, 5 * H], bf16, isOutput=False)
    bias = nc.declare_dram_parameter("bias", [128, 20], f32, isOutput=False)
    w2 = nc.declare_dram_parameter("w2", [128, 8 * H], bf16, isOutput=False)
    w3 = nc.declare_dram_parameter("w3", [128, 4 * H], bf16, isOutput=False)
    ot = nc.declare_dram_parameter("ot", [128, 4 * C], bf16, isOutput=True)

    m1 = H // 128      # 8 M-tiles for layers 1/2
    m3 = D_LAT // 128  # 4 M-tiles for layer 3
    ns = _n_slices(C)
    nj = len(ns)
    nsz = ns[0][1]

    with tile.TileContext(nc) as tc:
        with (
            tc.tile_pool(name="xw", bufs=1) as xw,
            tc.tile_pool(name="acts", bufs=1) as acts,
            tc.tile_pool(name="psum", bufs=8, space="PSUM") as psum,
        ):
            psp = [psum] * 8
            # Warmup operands (Tile requires every read tile to be written).
            wu_s = xw.tile([128, 128], bf16, name="wu_s")
            nc.vector.memset(wu_s[:], 0.0)
            wu_m = xw.tile([128, 512], bf16, name="wu_m")
            nc.vector.memset(wu_m[:], 0.0)

            # --- input DMAs -------------------------------------------------
            # ALL bandwidth-heavy inputs go on ONE queue (sync), strictly in
            # consumption order: the 16 SDMA engines round-robin between
            # queues at packet granularity, so a second busy queue steals
            # bandwidth from the JIT-critical stream. Each dma_start costs
            # ~0.6us issue + ~1.3us completion-receipt before its semaphore
            # fires, so x(j=0) is split per K-chunk for an early L1 start.
            # bias rides the scalar queue (one issue; queue then stays free
            # for evictions).
            bias_t = xw.tile([128, 20], f32, name="bias_t")
            nc.scalar.dma_start(out=bias_t[:], in_=bias[:])
            xn_t = [None] * nj
            xn_t[0] = xw.tile([128, 5 * nsz], bf16, name="xn_0")
            nc.sync.dma_start(out=xn_t[0][:, :nsz], in_=xn[:, :nsz])
            w1_t = []
            t = xw.tile([128, H], bf16, name="w1_0")
            nc.sync.dma_start(out=t[:], in_=w1[:, :H])
            w1_t.append(t)
            nc.sync.dma_start(
                out=xn_t[0][:, nsz : 2 * nsz], in_=xn[:, nsz : 2 * nsz]
            )
            t = xw.tile([128, H], bf16, name="w1_1")
            nc.sync.dma_start(out=t[:], in_=w1[:, H : 2 * H])
            w1_t.append(t)
            nc.sync.dma_start(
                out=xn_t[0][:, 2 * nsz : 5 * nsz], in_=xn[:, 2 * nsz : 5 * nsz]
            )
            for k in range(2, 5):
                t = xw.tile([128, H], bf16, name=f"w1_{k}")
                nc.sync.dma_start(out=t[:], in_=w1[:, k * H : (k + 1) * H])
                w1_t.append(t)
            for j in range(1, nj):
                xn_t[j] = xw.tile([128, 5 * nsz], bf16, name=f"xn_{j}")
                nc.sync.dma_start(
                    out=xn_t[j][:],
                    in_=xn[:, j * 5 * nsz : (j + 1) * 5 * nsz],
                )
            w2_t = []
            for i in range(4):
                t = xw.tile([128, 2 * H], bf16, name=f"w2_{i}")
                nc.sync.dma_start(out=t[:], in_=w2[:, i * 2 * H : (i + 1) * 2 * H])
                w2_t.append(t)
            w3_t = []
            for i in range(2):
                t = xw.tile([128, 2 * H], bf16, name=f"w3_{i}")
                nc.sync.dma_start(out=t[:], in_=w3[:, i * 2 * H : (i + 1) * 2 * H])
                w3_t.append(t)

            def w2_at(m, k):
                return w2_t[m // 2][
                    :, (m % 2) * H + k * 128 : (m % 2) * H + (k + 1) * 128
                ]

            def w3_at(m, k):
                return w3_t[m // 2][
                    :, (m % 2) * H + k * 128 : (m % 2) * H + (k + 1) * 128
                ]

            # Warmup: bf16 matmuls with no data dependencies heat the PE
            # clock gate (HAM) while the first chunks stream in.
            wu_p = psp[7].tile([128, 512], f32, tag="ps", name="wu_p")
            for _ in range(8):
                nc.tensor.matmul(
                    wu_p[:], lhsT=wu_s[:], rhs=wu_m[:], start=True, stop=True
                )

            # Inter-layer tiles are split per (n-chunk, M-tile) so consumers
            # depend only on the slice actually written (Tile tracks deps at
            # tile granularity).
            h1_t = [
                [acts.tile([128, nsz], bf16, name=f"h1_{j}_{m}") for m in range(m1)]
                for j in range(nj)
            ]
            h2_t = [
                [acts.tile([128, nsz], bf16, name=f"h2_{j}_{m}") for m in range(m1)]
                for j in range(nj)
            ]
            o_t = [
                [acts.tile([128, nsz], bf16, name=f"o_{j}_{m}") for m in range(m3)]
                for j in range(nj)
            ]

            ev_n = [0]

            def evict(out_ap, ps, bias_col, relu):
                """PSUM->SBUF eviction with bias (+relu), alternating
                ScalarE / VectorE so evictions never pace the PE."""
                b = bias_t[:, bias_col : bias_col + 1]
                if ev_n[0] % 2 == 0:
                    nc.scalar.activation(
                        out_ap, ps, AF.Relu if relu else AF.Identity, bias=b
                    )
                else:
                    if relu:
                        nc.vector.tensor_scalar(
                            out_ap, ps, b, 0.0, ALU.add, ALU.max
                        )
                    else:
                        nc.vector.tensor_scalar_add(out_ap, ps, b)
                ev_n[0] += 1

            # --- L1, n-chunk 0: K-outer / M-inner (JIT on w1 chunks) -------
            ps1 = [
                psp[m].tile([128, nsz], f32, tag="ps", name=f"ps1_{m}")
                for m in range(m1)
            ]
            for k in range(5):
                for m in range(m1):
                    nc.tensor.matmul(
                        ps1[m][:],
                        lhsT=w1_t[k][:, m * 128 : (m + 1) * 128],
                        rhs=xn_t[0][:, k * nsz : (k + 1) * nsz],
                        start=(k == 0),
                        stop=(k == 4),
                    )
            for m in range(m1):
                evict(h1_t[0][m][:], ps1[m][:], m, True)

            # --- L1, n-chunks 1+: M-outer / K-inner ------------------------
            for j in range(1, nj):
                for m in range(m1):
                    ps = psp[m].tile([128, nsz], f32, tag="ps", name=f"psa_{j}_{m}")
                    for k in range(5):
                        nc.tensor.matmul(
                            ps[:],
                            lhsT=w1_t[k][:, m * 128 : (m + 1) * 128],
                            rhs=xn_t[j][:, k * nsz : (k + 1) * nsz],
                            start=(k == 0),
                            stop=(k == 4),
                        )
                    evict(h1_t[j][m][:], ps[:], m, True)

            # --- L2: M-outer / K-inner per n-chunk -------------------------
            for j in range(nj):
                for m in range(m1):
                    ps = psp[m].tile([128, nsz], f32, tag="ps", name=f"psb_{j}_{m}")
                    for k in range(m1):
                        nc.tensor.matmul(
                            ps[:],
                            lhsT=w2_at(m, k),
                            rhs=h1_t[j][k][:],
                            start=(k == 0),
                            stop=(k == m1 - 1),
                        )
                    evict(h2_t[j][m][:], ps[:], 8 + m, True)

            # --- L3: M-outer / K-inner, flush each output immediately ------
            for j in range(nj):
                n0 = ns[j][0]
                for m in range(m3):
                    ps = psp[m].tile([128, nsz], f32, tag="ps", name=f"psc_{j}_{m}")
                    for k in range(m1):
                        nc.tensor.matmul(
                            ps[:],
                            lhsT=w3_at(m, k),
                            rhs=h2_t[j][k][:],
                            start=(k == 0),
                            stop=(k == m1 - 1),
                        )
                    evict(o_t[j][m][:], ps[:], 16 + m, False)
                    nc.sync.dma_start(
                        out=ot[:, m * C + n0 : m * C + n0 + nsz],
                        in_=o_t[j][m][:],
                    )

    nc.compile()
    return nc


def _ensure_axon_hooks():
    """run_bass_kernel_spmd(trace=True) imports antenv.axon_hooks, which the
    slim container lacks; provide it so tracing (e.g. BASS_TRACE=1) degrades
    gracefully or, if the ctypes hook is available, works."""
    import sys
    import types

    try:
        import antenv.axon_hooks  # noqa: F401
        return
    except ImportError:
        pass
    m = types.ModuleType("antenv.axon_hooks")
    m._hook = None
    m.set_axon_ntff_profile_hook = lambda h: setattr(m, "_hook", h)
    m.get_axon_ntff_profile_hook = lambda: m._hook
    sys.modules["antenv.axon_hooks"] = m
    try:
        from trn_agent_boot.trn_boot import _ntff_profile_via_ctypes

        m.set_axon_ntff_profile_hook(
            _ntff_profile_via_ctypes("/opt/axon/libaxon_pjrt.so")
        )
    except Exception:
        pass


def kernel(latents, actions, policy_indices, W1, b1, W2, b2, W3, b3):
    global LAST_RESULT
    _ensure_axon_hooks()
    from concourse.bass_utils import run_bass_kernel_spmd

    latents = np.ascontiguousarray(np.asarray(latents, dtype=np.float32))
    actions = np.ascontiguousarray(np.asarray(actions, dtype=np.float32))
    idx = np.asarray(policy_indices).astype(np.int64)
    W1 = np.asarray(W1, dtype=np.float32)
    b1 = np.asarray(b1, dtype=np.float32)
    W2 = np.asarray(W2, dtype=np.float32)
    b2 = np.asarray(b2, dtype=np.float32)
    W3 = np.asarray(W3, dtype=np.float32)
    b3 = np.asarray(b3, dtype=np.float32)

    n = latents.shape[0]
    order = np.argsort(idx, kind="stable")
    counts = np.bincount(idx, minlength=P)

    C = max(512, int(-(-counts.max() // 32)) * 32)
    k = -(-C // 512)
    C = -(-C // (16 * k)) * (16 * k)  # equal n-slices, width multiple of 16
    if C not in _compiled:
        _compiled[C] = _build(C)
    nc = _compiled[C]

    x = np.concatenate([latents, actions], axis=1)  # [B, 528]

    in_maps = []
    starts = np.concatenate([[0], np.cumsum(counts)])
    nsl = _n_slices(C)
    for p in range(P):
        sel = order[starts[p] : starts[p + 1]]
        xp = np.zeros((D_IN_PAD, C), dtype=BF16)
        xp[:D_IN, : counts[p]] = np.ascontiguousarray(x[sel].T).astype(BF16)
        xnp = np.concatenate(
            [_pretile(xp[:, n0 : n0 + nsz]) for n0, nsz in nsl], axis=1
        )
        w1r = np.zeros((D_IN_PAD, H), dtype=BF16)
        w1r[:D_IN] = W1[p].astype(BF16)
        w2p = np.concatenate(
            [_pretile(W2[p][:, m * 128 : (m + 1) * 128].astype(BF16))
             for m in range(8)],
            axis=1,
        )
        w3p = np.concatenate(
            [_pretile(W3[p][:, m * 128 : (m + 1) * 128].astype(BF16))
             for m in range(4)],
            axis=1,
        )
        bp = np.concatenate(
            [
                b1[p].reshape(H // 128, 128).T,
                b2[p].reshape(H // 128, 128).T,
                b3[p].reshape(D_LAT // 128, 128).T,
            ],
            axis=1,
        )
        in_maps.append(
            {
                "xn": xnp,
                "w1": _pretile(w1r),
                "bias": np.ascontiguousarray(bp),
                "w2": w2p,
                "w3": w3p,
            }
        )

    res = run_bass_kernel_spmd(nc, in_maps, core_ids=list(range(P)))
    LAST_RESULT = res

    out = np.empty((n, D_LAT), dtype=np.float32)
    for p in range(P):
        sel = order[starts[p] : starts[p + 1]]
        # [128, 4, C] -> [D_LAT, C]
        op = (
            res.results[p]["ot"]
            .reshape(128, 4, C)
            .transpose(1, 0, 2)
            .reshape(D_LAT, C)
            .astype(np.float32)
        )
        out[sel] = op[:, : counts[p]].T
    return out
